# revision 16
# baseline (speedup 1.0000x reference)
"""Trainium2 Bass kernel for nn_CASCADES_v8_ResonantCore (moe_routing).

Computation (per batch b):
    centroid = 0.7*x[b,-1,:] + 0.3*mean_s(x[b])
    w = softmax(cos_sim(centroid, core_keys)/TEMP)      # [K]
    Lam = sum_k w[k] * core_pool[k]                     # [R,R]
    out[b] = ((x[b] @ V^T) @ Lam^T) @ U^T               # [S,D]

Strategy (8 cores, data-parallel over (batch, seq-half)):
  - K1 (read pass): each core reads its [2048, 4096] shard of x once,
    computes xV^T = V @ x^T (bf16 matmuls, fp32 accumulate) and the
    column sums of x (for the mean) via a matmul-transpose chain on PE.
  - Host: combines the 8 partial column sums, does the tiny routing math
    (cosine/softmax over 16 numbers), folds Lam into W = U @ Lam.
  - K2 (write pass): each core computes out = xV @ W^T and writes its
    [2048, 4096] shard of the output.
  Total HBM traffic is read-x + write-out = the memory roofline; the two
  passes are inherently serial because every output element depends on
  the full-sequence mean through the routing weights.
"""

import sys

sys.path.insert(0, "/opt/trn_rl_repo")

import contextlib

import ml_dtypes
import numpy as np

import concourse.bass as bass  # noqa: F401  (registers bass types)
import concourse.tile as tile
from concourse import bacc, mybir
from concourse.bass_utils import run_bass_kernel_spmd

BF16 = ml_dtypes.bfloat16

B, S, D, R, K = 4, 4096, 4096, 8, 4
NCORES = 8
SH = S // 2  # rows of x per core
EPS, TEMP = 1e-8, 0.05

_cache = {}


def build_k1(sh=SH, d=D, r=R):
    """Read pass: xs [sh, d] f32 -> xvt [r, sh] f32 (= V @ x^T), cs [1, d] f32.

    cs[0, j] = sum_s bf16(x)[s, j]  (strip partials accumulated in bf16 on
    GpSimd, cross-partition reduced once on PE at the end).
    """
    nstrip, nch = sh // 128, d // 128
    ngrp = nch // 4
    nc = bacc.Bacc("TRN2", target_bir_lowering=False, debug=False)
    xs = nc.dram_tensor("xs", [sh, d], mybir.dt.float32, kind="ExternalInput").ap()
    vt = nc.dram_tensor("vt", [128, nch * r], mybir.dt.bfloat16, kind="ExternalInput").ap()
    consts = nc.dram_tensor("consts", [128, 129], mybir.dt.bfloat16, kind="ExternalInput").ap()
    xvt_out = nc.dram_tensor("xvt", [4 * r, sh], mybir.dt.float32, kind="ExternalOutput").ap()
    cs_out = nc.dram_tensor("cs", [1, d], mybir.dt.float32, kind="ExternalOutput").ap()

    with tile.TileContext(nc) as tc:
        with contextlib.ExitStack() as ctx:
            cpool = ctx.enter_context(tc.tile_pool(name="consts", bufs=1))
            xpool = ctx.enter_context(tc.tile_pool(name="x", bufs=6))
            tpool = ctx.enter_context(tc.tile_pool(name="xT4", bufs=8))
            apool = ctx.enter_context(tc.tile_pool(name="acc", bufs=1))
            opool = ctx.enter_context(tc.tile_pool(name="outs", bufs=1))
            psT = ctx.enter_context(tc.tile_pool(name="psT", bufs=4, space="PSUM"))
            psX = ctx.enter_context(tc.tile_pool(name="psX", bufs=2, space="PSUM"))
            psR = ctx.enter_context(tc.tile_pool(name="psR", bufs=1, space="PSUM"))

            vt_sb = cpool.tile([128, nch * r], mybir.dt.bfloat16)
            nc.sync.dma_start(vt_sb[:], vt[:])
            const_sb = cpool.tile([128, 129], mybir.dt.bfloat16)
            nc.sync.dma_start(const_sb[:], consts[:])
            idn = const_sb[:, 0:128]
            ones = const_sb[:, 128:129]
            # 4 row-bands of 8 at partitions {0,32,64,96}; host sums the bands
            xvt_sb = opool.tile([128, sh], mybir.dt.float32)
            acc = apool.tile([128, d], mybir.dt.bfloat16)
            nc.vector.memset(acc[:], 0.0)

            q = d // 4
            for i in range(nstrip):
                # four quarter tiles: compute starts as each cast-DMA lands
                xqs = []
                for piece in range(4):
                    xq = xpool.tile([128, q], mybir.dt.bfloat16, tag=f"xq{piece}")
                    nc.gpsimd.dma_start(
                        xq[:], xs[i * 128:(i + 1) * 128, piece * q:(piece + 1) * q]
                    )
                    xqs.append(xq)
                    # strip accumulation for the column sums
                    nc.vector.tensor_add(
                        acc[:, piece * q:(piece + 1) * q],
                        acc[:, piece * q:(piece + 1) * q], xq[:],
                    )
                ps_xvt = psX.tile([128, 128], mybir.dt.float32, tag="psxvt")
                xT4s = []
                for g in range(ngrp):
                    psTt = psT.tile([128, 512], mybir.dt.float32, tag="psT")
                    for k in range(4):
                        c = 4 * g + k
                        xq = xqs[c // (nch // 4)]
                        cc = c % (nch // 4)
                        # transpose chunk: psT[:, k] = xc^T (matmul vs identity)
                        nc.tensor.matmul(
                            psTt[:, k * 128:(k + 1) * 128],
                            xq[:, cc * 128:(cc + 1) * 128], idn,
                            start=(k == 0), stop=(k == 3),
                        )
                    xT4 = tpool.tile([128, 512], mybir.dt.bfloat16, tag="xT4")
                    nc.any.tensor_copy(xT4[:], psTt[:])
                    xT4s.append(xT4)

                    def mm1(gg):
                        # 4 concurrent col-group matmuls: band k accumulates
                        # chunks c=4g+k over g.  One psum group per bank:
                        # start only on the very first write, stop on the last.
                        for k in range(4):
                            c = 4 * gg + k
                            nc.tensor.matmul(
                                ps_xvt[32 * k:32 * k + r, :],
                                vt_sb[:, c * r:(c + 1) * r],
                                xT4s[gg][:, k * 128:(k + 1) * 128],
                                start=(gg == 0),
                                stop=(gg == ngrp - 1),
                                tile_position=(0, 32 * k),
                            )

                    if g >= 1:
                        mm1(g - 1)  # one-group software pipeline skew
                mm1(ngrp - 1)
                for k in range(4):
                    nc.any.tensor_copy(
                        xvt_sb[32 * k:32 * k + r, i * 128:(i + 1) * 128],
                        ps_xvt[32 * k:32 * k + r, :],
                    )
                if i % 4 == 3 or i == nstrip - 1:
                    lo, hi = (i - i % 4) * 128, (i + 1) * 128
                    for k in range(4):
                        nc.sync.dma_start(
                            xvt_out[r * k:r * (k + 1), lo:hi],
                            xvt_sb[32 * k:32 * k + r, lo:hi],
                        )

            # cross-partition reduce of acc: cs[0, :] = ones^T @ acc
            cs_sb = opool.tile([1, d], mybir.dt.float32)
            for j in range(d // 512):
                psRt = psR.tile([1, 512], mybir.dt.float32, tag="psR")
                nc.tensor.matmul(
                    psRt[:], ones, acc[:, j * 512:(j + 1) * 512],
                    start=True, stop=True,
                )
                nc.any.tensor_copy(cs_sb[:, j * 512:(j + 1) * 512], psRt[:])
            nc.sync.dma_start(cs_out[:], cs_sb[:])
    nc.compile()
    return nc


def build_k2(sh=SH, d=D, r=R):
    """Write pass: out [sh, d] f32 = (xvt^T @ wt) with xvt [r, sh], wt [r, d] bf16."""
    nsx, ndj = sh // 128, d // 512
    nc = bacc.Bacc("TRN2", target_bir_lowering=False, debug=False)
    xvt = nc.dram_tensor("xvt", [r, sh], mybir.dt.bfloat16, kind="ExternalInput").ap()
    wt = nc.dram_tensor("wt", [r, d], mybir.dt.bfloat16, kind="ExternalInput").ap()
    out = nc.dram_tensor("out", [sh, d], mybir.dt.float32, kind="ExternalOutput").ap()

    with tile.TileContext(nc) as tc:
        with contextlib.ExitStack() as ctx:
            cpool = ctx.enter_context(tc.tile_pool(name="consts", bufs=1))
            opool = ctx.enter_context(tc.tile_pool(name="ob", bufs=4))
            psP = ctx.enter_context(tc.tile_pool(name="ps2", bufs=3, space="PSUM"))

            xvt_sb = cpool.tile([r, sh], mybir.dt.bfloat16)
            nc.sync.dma_start(xvt_sb[:], xvt[:])
            wt_sb = cpool.tile([r, d], mybir.dt.bfloat16)
            nc.sync.dma_start(wt_sb[:], wt[:])

            for i in range(nsx):
                ob = opool.tile([128, d], mybir.dt.float32, tag="ob")
                for j in range(ndj // 2):
                    ps2 = psP.tile([128, 1024], mybir.dt.float32, tag="ps2")
                    for half in range(2):
                        nc.tensor.matmul(
                            ps2[:, half * 512:(half + 1) * 512],
                            xvt_sb[:, i * 128:(i + 1) * 128],
                            wt_sb[:, (2 * j + half) * 512:(2 * j + half + 1) * 512],
                            start=True, stop=True,
                        )
                    nc.any.tensor_copy(ob[:, j * 1024:(j + 1) * 1024], ps2[:])
                    if j == ndj // 4 - 1:
                        nc.sync.dma_start(
                            out[i * 128:(i + 1) * 128, :d // 2], ob[:, :d // 2]
                        )
                nc.sync.dma_start(out[i * 128:(i + 1) * 128, d // 2:], ob[:, d // 2:])
    nc.compile()
    return nc


def _get_kernels():
    if "k1" not in _cache:
        _cache["k1"] = build_k1()
        _cache["k2"] = build_k2()
    return _cache["k1"], _cache["k2"]


def _vt_layout(V, d, r):
    """[128, (d//128)*r] bf16 with vt[p, c*r + j] = V[j, c*128 + p]."""
    nch = d // 128
    # V [r, d] -> [r, nch, 128] -> [128, nch, r]
    return np.ascontiguousarray(
        V.reshape(r, nch, 128).transpose(2, 1, 0).reshape(128, nch * r)
    ).astype(BF16)


def _consts_layout():
    c = np.zeros((128, 129), dtype=np.float32)
    c[:, 0:128] = np.eye(128, dtype=np.float32)
    c[:, 128] = 1.0
    return c.astype(BF16)


def kernel(x, V_shared, U_shared, core_pool, core_keys):
    x = np.asarray(x)
    V_shared = np.asarray(V_shared)
    U_shared = np.asarray(U_shared)
    core_pool = np.asarray(core_pool)
    core_keys = np.asarray(core_keys)

    nc1, nc2 = _get_kernels()
    core_ids = list(range(NCORES))

    vt_np = _vt_layout(V_shared.astype(np.float32), D, R)
    consts_np = _consts_layout()

    in_maps1 = []
    for c in core_ids:
        b, h = c // 2, c % 2
        xs = np.ascontiguousarray(x[b, h * SH:(h + 1) * SH, :], dtype=np.float32)
        in_maps1.append({"xs": xs, "vt": vt_np, "consts": consts_np})
    res1 = run_bass_kernel_spmd(nc1, in_maps1, core_ids).results

    # --- host routing (tiny: 16 numbers through softmax) ---
    cs = [res1[c]["cs"].astype(np.float64).ravel() for c in core_ids]  # [d]
    # xvt comes back as 4 row-bands of r; sum them
    xvt = [
        res1[c]["xvt"].reshape(4, R, SH).sum(axis=0).astype(np.float32)
        for c in core_ids
    ]  # [r, SH]

    wt_b = []
    for b in range(B):
        colsum = cs[2 * b] + cs[2 * b + 1]
        mean = colsum / S
        centroid = 0.7 * x[b, -1, :].astype(np.float64) + 0.3 * mean
        c_n = centroid / max(np.linalg.norm(centroid), EPS)
        kk = core_keys.astype(np.float64)
        k_n = kk / np.maximum(np.linalg.norm(kk, axis=-1, keepdims=True), EPS)
        sim = c_n @ k_n.T  # [K]
        logits = sim / TEMP
        e = np.exp(logits - logits.max())
        w = e / e.sum()
        Lam = np.einsum("k,kij->ij", w, core_pool.astype(np.float64))  # [R, R]
        W = U_shared.astype(np.float64) @ Lam  # [D, R]
        wt_b.append(np.ascontiguousarray(W.T).astype(BF16))  # [R, D]

    in_maps2 = []
    for c in core_ids:
        b = c // 2
        in_maps2.append({"xvt": xvt[c].astype(BF16), "wt": wt_b[b]})
    res2 = run_bass_kernel_spmd(nc2, in_maps2, core_ids).results

    out = np.empty((B, S, D), dtype=np.float32)
    for c in core_ids:
        b, h = c // 2, c % 2
        out[b, h * SH:(h + 1) * SH, :] = res2[c]["out"]
    return out
